# revision 20
# baseline (speedup 1.0000x reference)
"""Trainium2 Bass kernel for the time-binned MoE EmbeddingClassifier.

Model: 11 expert MLPs (1536 -> 3072 -> 3072 -> 5242, exact GELU between
layers, log_softmax output). Each sample is routed to one expert by
bin = trunc((1 - mask_frac) / 0.1).

Strategy (8 NeuronCores, expert-parallel with host-side routing):
  - Routing is computed on the host from mask_frac; samples are grouped by
    expert. Only the routed expert runs per sample (11x less compute than
    the reference's run-all-then-select).
  - Experts 0..7 are whole-expert assigned to cores 0..7.
  - Experts 8 and 9 are each split 4 ways along the hidden dimension
    (cores 0-3 handle expert 8, cores 4-7 handle expert 9): each core
    computes the full layer 1, a 768-column slice of layer 2, and the
    matching 768-row slice of layer 3, producing a full-width partial
    logit sum. The host adds the 4 partials + b3 and applies log_softmax.
    This balances HBM weight traffic at ~83 MB/core (vs 121 MB for a
    naive 2-experts-on-one-core split).
  - Precision: x / W1 / activations in fp16 (same HBM bytes and PE rate
    as bf16 but 8x finer mantissa on this small-range data); W2/W3 in
    e4m3 fp8 with a x64 power-of-2 pre-scale (|W|~0.02 sits in e4m3's
    subnormal range) and the descale folded into the existing
    PSUM-drain ACT ops. Accumulation stays fp32 in PSUM; the logits and
    log_softmax stay fp32. Measured vs the fp32 reference:
    L2 rel err ~1.6e-3, absmax ~0.07 (vs |out| ~ 9-15).
  - Expert 10 (hit only when mask_frac == 0.0 exactly) and any samples
    beyond the per-expert capacity of 128 are computed on the host in
    fp32 as a correctness fallback; for the expected input distribution
    (~102 samples/expert) this never triggers.

Device layout: activations ride the partition dim as [samples<=128, feat];
weights stream as the moving matmul operand. Weights are host-packed into
per-output-chunk column blocks ([128, nk*cw] per 512-wide output chunk) so
the k-loop accumulates into a single PSUM bank with back-to-back matmuls
(no PSUM bank cycling -> PE stays pipelined and HAM-warm), and each block
arrives via ~1 MB DMA pieces so the PE never starves past the ~3.4 us HAM
re-throttle window. Between layers the activations are transposed 128x128
via the PE.
"""

import os
import sys

if "/opt/trn_rl_repo" not in sys.path:
    sys.path.insert(0, "/opt/trn_rl_repo")

import numpy as np
import ml_dtypes

import concourse.bass as bass
import concourse.tile as tile
from concourse import bacc, mybir
from concourse.bass_utils import run_bass_kernel_spmd

# half dtype for x / non-fp8 weights / activations: fp16 beats bf16 here --
# same bytes and PE rate, but 8x finer mantissa on this small-range data
MOE_HALF = os.environ.get("MOE_HALF", "fp16")
BF16 = mybir.dt.float16 if MOE_HALF == "fp16" else mybir.dt.bfloat16
FP8 = mybir.dt.float8e4
F32 = mybir.dt.float32
AF = mybir.ActivationFunctionType
NBF = np.float16 if MOE_HALF == "fp16" else ml_dtypes.bfloat16
NF8 = ml_dtypes.float8_e4m3
FP8_SCALE = 64.0     # power-of-2 pre-scale: |W|~0.02 sits in e4m3's subnormal
                     # range, x64 recenters it; descale rides the ACT op
# which layers stream fp8 weights ("l1"/"l2"/"l3"); overridable for A/B runs
FP8_LAYERS = frozenset(os.environ.get("MOE_FP8", "l2,l3").replace(",", " ").split())
# DoubleRow perf mode for the fp8 layers 2/3 (needs fp8 activations too)
MOE_DR = os.environ.get("MOE_DR", "0") == "1"
# transpose path: "pe" (tensor-engine + DVE drain) or "dma" (xbar transpose)
MOE_TR = os.environ.get("MOE_TR", "pe")
DRMODE = mybir.MatmulPerfMode.DoubleRow

E = 11
D = 1536
H = 3072
C = 5242
B = 1024
CAP = 128            # per-expert sample capacity on device
CPAD = 5248          # C padded to a multiple of 128 (10x512 + 128)
CMAIN = 5120         # first 10 layer-3 chunks (512 wide)
NK1 = D // 128       # 12 k-tiles for layer 1
NK2 = H // 128       # 24 k-tiles for layers 2/3
QCOLS = H // 4       # 768-wide hidden slice for the split experts
PAD_BIAS = -100.0    # b3 value for padded logit columns -> exp() == 0

LAST_RESULTS = None  # BassKernelResults of the most recent run (for test.py)

_NC_CACHE = {}


def _chunk_mm(nc, wpool, pspool, lhs_full, nk, wdram, jrow, cw, npieces,
              name, wdt=BF16, final_stop=False, dr=False):
    """Accumulate one [128, cw] output chunk over nk k-tiles into one PSUM
    tile. Weight block [128, nk*cw] is DMAed in npieces k-contiguous pieces
    (subtile deps let early matmuls start before the whole block lands)."""
    psum = pspool.tile([128, 512], F32, tag="acc", name=f"ps_{name}")
    wblk = wpool.tile([128, nk * cw], wdt, tag="wblk", name=f"wb_{name}")
    cols = nk * cw
    kg = nk // npieces
    for pc in range(npieces):
        c0 = pc * kg * cw
        c1 = cols if pc == npieces - 1 else (pc + 1) * kg * cw
        nc.sync.dma_start(wblk[:, c0:c1],
                          wdram[jrow * 128:(jrow + 1) * 128, c0:c1])
    if dr:
        for t in range(nk // 2):
            lhs = lhs_full[:, 256 * t:256 * (t + 1)].rearrange(
                "p (i m) -> p i m", i=2)
            rhs = wblk[:, 2 * cw * t:2 * cw * (t + 1)].rearrange(
                "p (i n) -> p i n", i=2)
            nc.tensor.matmul(psum[:, :cw], lhs, rhs, perf_mode=DRMODE,
                             start=(t == 0),
                             stop=(final_stop and t == nk // 2 - 1))
    else:
        for k in range(nk):
            nc.tensor.matmul(psum[:, :cw], lhs_full[:, k * 128:(k + 1) * 128],
                             wblk[:, k * cw:(k + 1) * cw],
                             start=(k == 0),
                             stop=(final_stop and k == nk - 1))
    return psum


def _transpose(nc, hpool, tppool, src, ncols, ident_t, name, hdt=BF16):
    """Transpose src [128, ncols] per 128-chunk -> new tile [128, ncols].
    MOE_TR="pe": tensor-engine transpose (bf16; fp8 needs stride-2 PSUM APs)
    + DVE drain-copy casting to hdt. MOE_TR="dma": xbar DMA transpose
    (2-byte dtypes only, so hdt must not be fp8)."""
    out = hpool.tile([128, H], hdt, tag="h", name=f"t_{name}")
    if MOE_TR == "dma" and hdt is not FP8:
        for k in range(ncols // 128):
            nc.scalar.dma_start_transpose(out[:, k * 128:(k + 1) * 128],
                                          src[:, k * 128:(k + 1) * 128])
        return out
    for k in range(ncols // 128):
        tp = tppool.tile([128, 128], BF16, tag="tp", name=f"tp_{name}_{k}")
        nc.tensor.transpose(tp[:], src[:, k * 128:(k + 1) * 128], ident_t[:])
        nc.vector.tensor_copy(out[:, k * 128:(k + 1) * 128], tp[:])
    return out


def _mlp_unit(nc, pools, xs, w1cb, w2cb, w3cb, w3cbl, b1s, b2s, b3s, ones_t,
              ident_t, ident8_t, out_ap, nk3, ncols2, cw2, whole, uname,
              with_bias=True):
    """One expert unit: x -> gelu -> gelu -> logits [-> log_softmax] -> out."""
    hpool, wpool, zpool, epool, spool, pspool, tppool = pools
    dt1 = FP8 if "l1" in FP8_LAYERS else BF16
    dt2 = FP8 if "l2" in FP8_LAYERS else BF16
    dt3 = FP8 if "l3" in FP8_LAYERS else BF16
    sc1 = 1.0 / FP8_SCALE if dt1 is FP8 else 1.0
    sc2 = 1.0 / FP8_SCALE if dt2 is FP8 else 1.0
    sc3 = 1.0 / FP8_SCALE if dt3 is FP8 else 1.0
    # DoubleRow: fp8 weights AND fp8 activations for layers 2/3
    hdt = FP8 if MOE_DR else BF16
    tid = ident_t

    # ---- layer 1: h1[s, h] = gelu(x @ W1 + b1), 6 chunks of 512
    h1 = hpool.tile([128, H], BF16, tag="h", name=f"h1_{uname}")
    for j in range(H // 512):
        ps = _chunk_mm(nc, wpool, pspool, xs, NK1, w1cb, j, 512, 2,
                       f"{uname}l1j{j}", wdt=dt1, final_stop=not with_bias)
        if with_bias:
            nc.tensor.matmul(ps[:], ones_t[:], b1s[:, j * 512:(j + 1) * 512],
                             start=False, stop=True)
        nc.scalar.activation(h1[:, j * 512:(j + 1) * 512], ps[:], AF.Gelu,
                             scale=sc1)

    h1t = _transpose(nc, hpool, tppool, h1, H, tid, f"h1_{uname}", hdt=hdt)

    # ---- layer 2: h2[s, c] = gelu(h1 @ W2 + b2) over ncols2 cols
    h2 = hpool.tile([128, H], BF16, tag="h", name=f"h2_{uname}")
    for j in range(ncols2 // cw2):
        ps = _chunk_mm(nc, wpool, pspool, h1t, NK2, w2cb, j, cw2, 3,
                       f"{uname}l2j{j}", wdt=dt2, dr=MOE_DR,
                       final_stop=not with_bias)
        if with_bias:
            nc.tensor.matmul(ps[:, :cw2], ones_t[:],
                             b2s[:, j * cw2:(j + 1) * cw2],
                             start=False, stop=True)
        nc.scalar.activation(h2[:, j * cw2:(j + 1) * cw2], ps[:, :cw2],
                             AF.Gelu, scale=sc2)

    h2t = _transpose(nc, hpool, tppool, h2, ncols2, tid, f"h2_{uname}", hdt=hdt)

    # ---- layer 3: z[s, c] = h2 @ W3 (+ b3), 10x512 + 1x128 chunks
    z = zpool.tile([128, CPAD], F32, tag="z", name=f"z_{uname}")
    l3 = [(j, j * 512, 512, w3cb, j, 3 if nk3 == NK2 else 1)
          for j in range(CMAIN // 512)]
    l3.append((10, CMAIN, 128, w3cbl, 0, 1))
    for j, c0, cw, wdram, jrow, npieces in l3:
        ps = _chunk_mm(nc, wpool, pspool, h2t, nk3, wdram, jrow, cw, npieces,
                       f"{uname}l3j{j}", wdt=dt3,
                       final_stop=(not whole) or (not with_bias), dr=MOE_DR)
        if whole and with_bias:
            nc.tensor.matmul(ps[:, :cw], ones_t[:], b3s[:, c0:c0 + cw],
                             start=False, stop=True)
        nc.scalar.activation(z[:, c0:c0 + cw], ps[:, :cw], AF.Copy,
                             bias=0.0, scale=sc3)
        if not whole:
            nc.scalar.dma_start(out_ap[:, c0:c0 + cw], z[:, c0:c0 + cw])

    if whole:
        # ---- log_softmax along the free axis (pad cols hold z = -100)
        ncz = CPAD // 512 + 1
        s = spool.tile([128, ncz], F32, tag="s", name=f"s_{uname}")
        for j in range(ncz):
            c0 = j * 512
            cw = min(512, CPAD - c0)
            e_scr = epool.tile([128, 512], BF16, tag="e",
                               name=f"e_{uname}_{j}")
            nc.scalar.activation(e_scr[:, :cw], z[:, c0:c0 + cw], AF.Exp,
                                 accum_out=s[:, j:j + 1])
        stot = spool.tile([128, 1], F32, tag="stot", name=f"st_{uname}")
        nc.vector.tensor_reduce(stot[:], s[:], mybir.AxisListType.X,
                                mybir.AluOpType.add)
        if not with_bias:
            nc.vector.tensor_scalar_add(stot[:], stot[:], float(C - CPAD))
        lse = spool.tile([128, 1], F32, tag="lse", name=f"lse_{uname}")
        nc.scalar.activation(lse[:], stot[:], AF.Ln)
        for j in range(ncz):
            c0 = j * 512
            cw = min(512, CPAD - c0)
            nc.vector.tensor_scalar_sub(z[:, c0:c0 + cw], z[:, c0:c0 + cw],
                                        lse[:])
            nc.scalar.dma_start(out_ap[:, c0:c0 + cw], z[:, c0:c0 + cw])


def _build_nc(with_bias=True):
    nc = bacc.Bacc("TRN2", target_bir_lowering=False, debug=False,
                   num_devices=8)

    def din(name, shape, dt=BF16):
        return nc.dram_tensor(name, shape, dt, kind="ExternalInput").ap()

    xw = din("xw", [128, D])
    xq = din("xq", [128, D])
    dt1 = FP8 if "l1" in FP8_LAYERS else BF16
    dt2 = FP8 if "l2" in FP8_LAYERS else BF16
    dt3 = FP8 if "l3" in FP8_LAYERS else BF16
    w1cb = din("w1cb", [6 * 128, NK1 * 512], dt1)
    w2cb = din("w2cb", [6 * 128, NK2 * 512], dt2)
    w3cb = din("w3cb", [10 * 128, NK2 * 512], dt3)
    w3cbl = din("w3cbl", [128, NK2 * 128], dt3)
    w1qcb = din("w1qcb", [6 * 128, NK1 * 512], dt1)
    w2qcb = din("w2qcb", [2 * 128, NK2 * 384], dt2)
    w3qcb = din("w3qcb", [10 * 128, 6 * 512], dt3)
    w3qcbl = din("w3qcbl", [128, 6 * 128], dt3)
    if with_bias:
        b1w = din("b1w", [1, H])
        b2w = din("b2w", [1, H])
        b3w = din("b3w", [1, CPAD])
        b1q = din("b1q", [1, H])
        b2q = din("b2q", [1, QCOLS])
    ones = din("ones", [1, 128])
    ident = din("ident", [128, 128])
    ident8 = din("ident8", [128, 128], FP8)
    outw = nc.dram_tensor("outw", [128, CPAD], F32, kind="ExternalOutput").ap()
    outq = nc.dram_tensor("outq", [128, CPAD], F32, kind="ExternalOutput").ap()

    wbufs = 7 if ("l2" in FP8_LAYERS and "l3" in FP8_LAYERS) else 4
    with tile.TileContext(nc) as tc:
        with tc.tile_pool(name="hp", bufs=5) as hpool, \
             tc.tile_pool(name="wp", bufs=wbufs) as wpool, \
             tc.tile_pool(name="zp", bufs=2) as zpool, \
             tc.tile_pool(name="ep", bufs=2) as epool, \
             tc.tile_pool(name="sp", bufs=2) as spool, \
             tc.tile_pool(name="cp", bufs=1) as cpool, \
             tc.tile_pool(name="ps", bufs=5, space="PSUM") as pspool, \
             tc.tile_pool(name="tp", bufs=3 if "l3" in FP8_LAYERS else 2, space="PSUM") as tppool:
            pools = (hpool, wpool, zpool, epool, spool, pspool, tppool)

            # x first (first matmuls need it), consts on the ACT HWDGE queue
            # so they don't delay the weight-block stream on the SP queue
            xw_t = cpool.tile([128, D], BF16, tag="xw")
            nc.sync.dma_start(xw_t[:], xw)
            ones_t = cpool.tile([1, 128], BF16, tag="ones")
            nc.scalar.dma_start(ones_t[:], ones)
            ident_t = cpool.tile([128, 128], BF16, tag="ident")
            nc.scalar.dma_start(ident_t[:], ident)
            xq_t = cpool.tile([128, D], BF16, tag="xq")
            nc.scalar.dma_start(xq_t[:], xq)
            if with_bias:
                b1w_t = cpool.tile([1, H], BF16, tag="b1w")
                nc.scalar.dma_start(b1w_t[:], b1w)
                b2w_t = cpool.tile([1, H], BF16, tag="b2w")
                nc.scalar.dma_start(b2w_t[:], b2w)
                b3w_t = cpool.tile([1, CPAD], BF16, tag="b3w")
                nc.scalar.dma_start(b3w_t[:], b3w)
                b1q_t = cpool.tile([1, H], BF16, tag="b1q")
                nc.scalar.dma_start(b1q_t[:], b1q)
                b2q_t = cpool.tile([1, QCOLS], BF16, tag="b2q")
                nc.scalar.dma_start(b2q_t[:], b2q)
            else:
                b1w_t = b2w_t = b3w_t = b1q_t = b2q_t = None
            ident8_t = cpool.tile([128, 128], FP8, tag="ident8")
            nc.scalar.dma_start(ident8_t[:], ident8)

            _mlp_unit(nc, pools, xw_t[:], w1cb, w2cb, w3cb, w3cbl,
                      b1w_t, b2w_t, b3w_t, ones_t, ident_t, ident8_t, outw,
                      nk3=NK2, ncols2=H, cw2=512, whole=True, uname="w",
                      with_bias=with_bias)
            _mlp_unit(nc, pools, xq_t[:], w1qcb, w2qcb, w3qcb, w3qcbl,
                      b1q_t, b2q_t, None, ones_t, ident_t, ident8_t, outq,
                      nk3=QCOLS // 128, ncols2=QCOLS, cw2=384, whole=False,
                      uname="q", with_bias=with_bias)
    nc.compile()
    return nc


def _cb_pack(W, cw, layer):
    """[K, Ctot] -> per-cw-chunk column blocks [nch*128, nk*cw] where
    block row p, col k*cw + c = W[k*128 + p, j*cw + c]. In DoubleRow mode
    (fp8 l2/l3) rows pair up per 256-super: col t*2cw + i*cw + c maps to
    row 256t + 128i + p."""
    K, Ct = W.shape
    nk, nch = K // 128, Ct // cw
    if layer in FP8_LAYERS:
        ndt = NF8
        Wr = (np.asarray(W, dtype=np.float32) * FP8_SCALE).astype(NF8)
    else:
        ndt = NBF
        Wr = np.asarray(W, dtype=NBF)
    Wr = Wr.reshape(nk, 128, Ct)
    out = np.empty((nch * 128, nk * cw), dtype=ndt)
    for j in range(nch):
        blk = Wr[:, :, j * cw:(j + 1) * cw]        # [nk, 128, cw]
        if MOE_DR and layer in ("l2", "l3"):
            # [t, i, p, c] -> [p, t, i, c] -> cols ordered t*2cw + i*cw + c
            out[j * 128:(j + 1) * 128] = (
                blk.reshape(nk // 2, 2, 128, cw).transpose(2, 0, 1, 3)
                .reshape(128, nk * cw))
        else:
            out[j * 128:(j + 1) * 128] = (
                blk.transpose(1, 0, 2).reshape(128, nk * cw))
    return out


def _erf(v):
    try:
        from scipy.special import erf
        return erf(v)
    except ImportError:
        import math
        return np.vectorize(math.erf)(v)


def _host_expert(x_rows, W1e, b1e, W2e, b2e, W3e, b3e):
    """fp32 numpy fallback, mirroring the reference exactly."""

    def gelu(v):
        return (v * 0.5 * (1.0 + _erf(v / np.sqrt(2.0)))).astype(np.float32)

    h1 = gelu(x_rows @ W1e + b1e)
    h2 = gelu(h1 @ W2e + b2e)
    z = (h2 @ W3e + b3e).astype(np.float64)
    m = z.max(axis=1, keepdims=True)
    lse = np.log(np.exp(z - m).sum(axis=1, keepdims=True)) + m
    return (z - lse).astype(np.float32)


def kernel(x, mask_frac, W1, b1, W2, b2, W3, b3):
    global LAST_RESULTS, _NC_CACHE

    x = np.asarray(x, dtype=np.float32)
    mask_frac = np.asarray(mask_frac, dtype=np.float32)
    W1 = np.asarray(W1, dtype=np.float32)
    b1 = np.asarray(b1, dtype=np.float32)
    W2 = np.asarray(W2, dtype=np.float32)
    b2 = np.asarray(b2, dtype=np.float32)
    W3 = np.asarray(W3, dtype=np.float32)
    b3 = np.asarray(b3, dtype=np.float32)

    # host routing, mirroring the reference's fp32 arithmetic
    t = np.float32(1.0) - mask_frac
    bins = (t / np.float32(0.1)).astype(np.int32)

    with_bias = bool(b1.any() or b2.any() or b3.any())

    groups = [np.where(bins == e)[0] for e in range(E)]
    fallback = []  # (expert, sample indices) pairs computed on host
    dev_groups = []
    for e in range(10):
        idx = groups[e]
        if len(idx) > CAP:
            fallback.append((e, idx[CAP:]))
            idx = idx[:CAP]
        dev_groups.append(idx)
    if len(groups[10]):
        fallback.append((10, groups[10]))

    def pack_x(idx):
        # [128, D] bf16 with xs[p, k*128 + n] = x[idx[n], k*128 + p]
        xt = np.zeros((128, D), dtype=NBF)
        if len(idx):
            xe = x[idx].astype(NBF)            # [n, D]
            xr = np.ascontiguousarray(
                xe.reshape(len(idx), NK1, 128).transpose(2, 1, 0))
            xt.reshape(128, NK1, 128)[:, :, :len(idx)] = xr
        return xt

    bsc1 = FP8_SCALE if "l1" in FP8_LAYERS else 1.0
    bsc2 = FP8_SCALE if "l2" in FP8_LAYERS else 1.0
    bsc3 = FP8_SCALE if "l3" in FP8_LAYERS else 1.0
    b3pad = np.full((1, CPAD), PAD_BIAS * bsc3, dtype=NBF)
    ones_np = np.ones((1, 128), dtype=NBF)
    ident_np = np.eye(128, dtype=NBF)
    ident8_np = np.eye(128, dtype=NF8)

    in_maps = []
    for c in range(8):
        q = 8 if c < 4 else 9          # split expert handled by this core
        qq = c % 4                     # hidden-dim quarter index
        b3row = b3pad.copy()
        b3row[0, :C] = (b3[c] * bsc3).astype(NBF)
        w3pad = np.zeros((H, CPAD), dtype=np.float32)
        w3pad[:, :C] = W3[c]
        w3qpad = np.zeros((QCOLS, CPAD), dtype=np.float32)
        w3qpad[:, :C] = W3[q][qq * QCOLS:(qq + 1) * QCOLS]
        bias_ins = {
            "b1w": (b1[c] * bsc1).astype(NBF).reshape(1, H),
            "b2w": (b2[c] * bsc2).astype(NBF).reshape(1, H),
            "b3w": b3row,
            "b1q": (b1[q] * bsc1).astype(NBF).reshape(1, H),
            "b2q": np.ascontiguousarray(
                (b2[q][qq * QCOLS:(qq + 1) * QCOLS] * bsc2).astype(NBF)
            ).reshape(1, QCOLS),
        } if with_bias else {}
        in_maps.append({
            **bias_ins,
            "xw": pack_x(dev_groups[c]),
            "xq": pack_x(dev_groups[q]),
            "w1cb": _cb_pack(W1[c], 512, "l1"),
            "w2cb": _cb_pack(W2[c], 512, "l2"),
            "w3cb": _cb_pack(w3pad[:, :CMAIN], 512, "l3"),
            "w3cbl": _cb_pack(w3pad[:, CMAIN:], 128, "l3"),
            "w1qcb": _cb_pack(W1[q], 512, "l1"),
            "w2qcb": _cb_pack(W2[q][:, qq * QCOLS:(qq + 1) * QCOLS], 384, "l2"),
            "w3qcb": _cb_pack(w3qpad[:, :CMAIN], 512, "l3"),
            "w3qcbl": _cb_pack(w3qpad[:, CMAIN:], 128, "l3"),
            "ones": ones_np,
            "ident": ident_np,
            "ident8": ident8_np,
        })

    if with_bias not in _NC_CACHE:
        _NC_CACHE[with_bias] = _build_nc(with_bias)
    res = run_bass_kernel_spmd(_NC_CACHE[with_bias], in_maps,
                               core_ids=list(range(8)))
    LAST_RESULTS = res

    out = np.zeros((B, C), dtype=np.float32)
    for c in range(8):
        idx = dev_groups[c]
        if len(idx):
            out[idx] = res.results[c]["outw"][:len(idx), :C]

    # split experts: host-sum the 4 hidden-quarter partials + b3, log_softmax
    for q, cores in ((8, (0, 1, 2, 3)), (9, (4, 5, 6, 7))):
        idx = dev_groups[q]
        if not len(idx):
            continue
        zsum = np.zeros((len(idx), C), dtype=np.float64)
        for c in cores:
            zsum += res.results[c]["outq"][:len(idx), :C]
        zsum += b3[q]
        m = zsum.max(axis=1, keepdims=True)
        lse = np.log(np.exp(zsum - m).sum(axis=1, keepdims=True)) + m
        out[idx] = (zsum - lse).astype(np.float32)

    for e, idx in fallback:
        out[idx] = _host_expert(x[idx], W1[e], b1[e], W2[e], b2[e],
                                W3[e], b3[e])
    return out


# revision 21
# speedup vs baseline: 1.7174x; 1.7174x over previous
"""Trainium2 Bass kernel for the time-binned MoE EmbeddingClassifier.

Model: 11 expert MLPs (1536 -> 3072 -> 3072 -> 5242, exact GELU between
layers, log_softmax output). Each sample is routed to one expert by
bin = trunc((1 - mask_frac) / 0.1).

Strategy (8 NeuronCores, expert-parallel with host-side routing):
  - Routing is computed on the host from mask_frac; samples are grouped by
    expert. Only the routed expert runs per sample (11x less compute than
    the reference's run-all-then-select).
  - Experts 0..7 are whole-expert assigned to cores 0..7.
  - Experts 8 and 9 are each split 4 ways along the hidden dimension
    (cores 0-3 handle expert 8, cores 4-7 handle expert 9): each core
    computes the full layer 1, a 768-column slice of layer 2, and the
    matching 768-row slice of layer 3, producing a full-width partial
    logit sum. The host adds the 4 partials + b3 and applies log_softmax.
    This balances HBM weight traffic at ~83 MB/core (vs 121 MB for a
    naive 2-experts-on-one-core split).
  - Precision: x / W1 / activations in fp16 (same HBM bytes and PE rate
    as bf16 but 8x finer mantissa on this small-range data); W2/W3 in
    e4m3 fp8 with a x64 power-of-2 pre-scale (|W|~0.02 sits in e4m3's
    subnormal range) and the descale folded into the existing
    PSUM-drain ACT ops. Accumulation stays fp32 in PSUM; the logits and
    log_softmax stay fp32. Measured vs the fp32 reference:
    L2 rel err ~1.6e-3, absmax ~0.07 (vs |out| ~ 9-15).
  - Expert 10 (hit only when mask_frac == 0.0 exactly) and any samples
    beyond the per-expert capacity of 128 are computed on the host in
    fp32 as a correctness fallback; for the expected input distribution
    (~102 samples/expert) this never triggers.

Device layout: activations ride the partition dim as [samples<=128, feat];
weights stream as the moving matmul operand. Weights are host-packed into
per-output-chunk column blocks ([128, nk*cw] per 512-wide output chunk) so
the k-loop accumulates into a single PSUM bank with back-to-back matmuls
(no PSUM bank cycling -> PE stays pipelined and HAM-warm), and each block
arrives via ~1 MB DMA pieces so the PE never starves past the ~3.4 us HAM
re-throttle window. Between layers the activations are transposed 128x128
via the PE.
"""

import os
import sys

if "/opt/trn_rl_repo" not in sys.path:
    sys.path.insert(0, "/opt/trn_rl_repo")

import numpy as np
import ml_dtypes

import concourse.bass as bass
import concourse.tile as tile
from concourse import bacc, mybir
from concourse.bass_utils import run_bass_kernel_spmd

# half dtype for x / non-fp8 weights / activations: fp16 beats bf16 here --
# same bytes and PE rate, but 8x finer mantissa on this small-range data
MOE_HALF = os.environ.get("MOE_HALF", "fp16")
BF16 = mybir.dt.float16 if MOE_HALF == "fp16" else mybir.dt.bfloat16
FP8 = mybir.dt.float8e4
F32 = mybir.dt.float32
AF = mybir.ActivationFunctionType
NBF = np.float16 if MOE_HALF == "fp16" else ml_dtypes.bfloat16
NF8 = ml_dtypes.float8_e4m3
FP8_SCALE = 64.0     # power-of-2 pre-scale: |W|~0.02 sits in e4m3's subnormal
                     # range, x64 recenters it; descale rides the ACT op
# which layers stream fp8 weights ("l1"/"l2"/"l3"); overridable for A/B runs
FP8_LAYERS = frozenset(os.environ.get("MOE_FP8", "l2,l3").replace(",", " ").split())
# DoubleRow perf mode for the fp8 layers 2/3 (needs fp8 activations too)
MOE_DR = os.environ.get("MOE_DR", "0") == "1"
# transpose path: "pe" (tensor-engine + DVE drain) or "dma" (xbar transpose)
MOE_TR = os.environ.get("MOE_TR", "pe")
DRMODE = mybir.MatmulPerfMode.DoubleRow

E = 11
D = 1536
H = 3072
C = 5242
B = 1024
CAP = 128            # per-expert sample capacity on device
CPAD = 5248          # C padded to a multiple of 128 (10x512 + 128)
CMAIN = 5120         # first 10 layer-3 chunks (512 wide)
NK1 = D // 128       # 12 k-tiles for layer 1
NK2 = H // 128       # 24 k-tiles for layers 2/3
QCOLS = H // 4       # 768-wide hidden slice for the split experts
PAD_BIAS = -100.0    # b3 value for padded logit columns -> exp() == 0

LAST_RESULTS = None  # BassKernelResults of the most recent run (for test.py)

_NC_CACHE = {}


def _chunk_mm(nc, wpool, pspool, lhs_full, nk, wdram, jrow, cw, npieces,
              name, wdt=BF16, final_stop=False, dr=False):
    """Accumulate one [128, cw] output chunk over nk k-tiles into one PSUM
    tile. Weight block [128, nk*cw] is DMAed in npieces k-contiguous pieces
    (subtile deps let early matmuls start before the whole block lands)."""
    psum = pspool.tile([128, 512], F32, tag="acc", name=f"ps_{name}")
    wblk = wpool.tile([128, nk * cw], wdt, tag="wblk", name=f"wb_{name}")
    cols = nk * cw
    kg = nk // npieces
    for pc in range(npieces):
        c0 = pc * kg * cw
        c1 = cols if pc == npieces - 1 else (pc + 1) * kg * cw
        nc.sync.dma_start(wblk[:, c0:c1],
                          wdram[jrow * 128:(jrow + 1) * 128, c0:c1])
    if dr:
        for t in range(nk // 2):
            lhs = lhs_full[:, 256 * t:256 * (t + 1)].rearrange(
                "p (i m) -> p i m", i=2)
            rhs = wblk[:, 2 * cw * t:2 * cw * (t + 1)].rearrange(
                "p (i n) -> p i n", i=2)
            nc.tensor.matmul(psum[:, :cw], lhs, rhs, perf_mode=DRMODE,
                             start=(t == 0),
                             stop=(final_stop and t == nk // 2 - 1))
    else:
        for k in range(nk):
            nc.tensor.matmul(psum[:, :cw], lhs_full[:, k * 128:(k + 1) * 128],
                             wblk[:, k * cw:(k + 1) * cw],
                             start=(k == 0),
                             stop=(final_stop and k == nk - 1))
    return psum


def _transpose(nc, hpool, tppool, src, ncols, ident_t, name, hdt=BF16):
    """Transpose src [128, ncols] per 128-chunk -> new tile [128, ncols].
    MOE_TR="pe": tensor-engine transpose (bf16; fp8 needs stride-2 PSUM APs)
    + DVE drain-copy casting to hdt. MOE_TR="dma": xbar DMA transpose
    (2-byte dtypes only, so hdt must not be fp8)."""
    out = hpool.tile([128, H], hdt, tag="h", name=f"t_{name}")
    if MOE_TR == "dma" and hdt is not FP8:
        for k in range(ncols // 128):
            nc.scalar.dma_start_transpose(out[:, k * 128:(k + 1) * 128],
                                          src[:, k * 128:(k + 1) * 128])
        return out
    for k in range(ncols // 128):
        tp = tppool.tile([128, 128], BF16, tag="tp", name=f"tp_{name}_{k}")
        nc.tensor.transpose(tp[:], src[:, k * 128:(k + 1) * 128], ident_t[:])
        nc.vector.tensor_copy(out[:, k * 128:(k + 1) * 128], tp[:])
    return out


def _mlp_unit(nc, pools, xs, w1cb, w2cb, w3cb, w3cbl, b1s, b2s, b3s, ones_t,
              ident_t, ident8_t, out_ap, nk3, ncols2, cw2, whole, uname,
              with_bias=True):
    """One expert unit: x -> gelu -> gelu -> logits [-> log_softmax] -> out."""
    hpool, wpool, zpool, epool, spool, pspool, tppool = pools
    dt1 = FP8 if "l1" in FP8_LAYERS else BF16
    dt2 = FP8 if "l2" in FP8_LAYERS else BF16
    dt3 = FP8 if "l3" in FP8_LAYERS else BF16
    sc1 = 1.0 / FP8_SCALE if dt1 is FP8 else 1.0
    sc2 = 1.0 / FP8_SCALE if dt2 is FP8 else 1.0
    sc3 = 1.0 / FP8_SCALE if dt3 is FP8 else 1.0
    # DoubleRow: fp8 weights AND fp8 activations for layers 2/3
    hdt = FP8 if MOE_DR else BF16
    tid = ident_t

    # ---- layer 1: h1[s, h] = gelu(x @ W1 + b1), 6 chunks of 512
    h1 = hpool.tile([128, H], BF16, tag="h", name=f"h1_{uname}")
    for j in range(H // 512):
        ps = _chunk_mm(nc, wpool, pspool, xs, NK1, w1cb, j, 512, 2,
                       f"{uname}l1j{j}", wdt=dt1, final_stop=not with_bias)
        if with_bias:
            nc.tensor.matmul(ps[:], ones_t[:], b1s[:, j * 512:(j + 1) * 512],
                             start=False, stop=True)
        nc.scalar.activation(h1[:, j * 512:(j + 1) * 512], ps[:], AF.Gelu,
                             scale=sc1)

    h1t = _transpose(nc, hpool, tppool, h1, H, tid, f"h1_{uname}", hdt=hdt)

    # ---- layer 2: h2[s, c] = gelu(h1 @ W2 + b2) over ncols2 cols
    h2 = hpool.tile([128, H], BF16, tag="h", name=f"h2_{uname}")
    for j in range(ncols2 // cw2):
        ps = _chunk_mm(nc, wpool, pspool, h1t, NK2, w2cb, j, cw2, 3,
                       f"{uname}l2j{j}", wdt=dt2, dr=MOE_DR,
                       final_stop=not with_bias)
        if with_bias:
            nc.tensor.matmul(ps[:, :cw2], ones_t[:],
                             b2s[:, j * cw2:(j + 1) * cw2],
                             start=False, stop=True)
        nc.scalar.activation(h2[:, j * cw2:(j + 1) * cw2], ps[:, :cw2],
                             AF.Gelu, scale=sc2)

    h2t = _transpose(nc, hpool, tppool, h2, ncols2, tid, f"h2_{uname}", hdt=hdt)

    # ---- layer 3: z[s, c] = h2 @ W3 (+ b3), 10x512 + 1x128 chunks
    z = zpool.tile([128, CPAD], F32, tag="z", name=f"z_{uname}")
    l3 = [(j, j * 512, 512, w3cb, j, 3 if nk3 == NK2 else 1)
          for j in range(CMAIN // 512)]
    l3.append((10, CMAIN, 128, w3cbl, 0, 1))
    for j, c0, cw, wdram, jrow, npieces in l3:
        ps = _chunk_mm(nc, wpool, pspool, h2t, nk3, wdram, jrow, cw, npieces,
                       f"{uname}l3j{j}", wdt=dt3,
                       final_stop=(not whole) or (not with_bias), dr=MOE_DR)
        if whole and with_bias:
            nc.tensor.matmul(ps[:, :cw], ones_t[:], b3s[:, c0:c0 + cw],
                             start=False, stop=True)
        nc.scalar.activation(z[:, c0:c0 + cw], ps[:, :cw], AF.Copy,
                             bias=0.0, scale=sc3)
        if not whole:
            nc.scalar.dma_start(out_ap[:, c0:c0 + cw], z[:, c0:c0 + cw])

    if whole:
        # ---- log_softmax along the free axis (pad cols hold z = -100)
        ncz = CPAD // 512 + 1
        s = spool.tile([128, ncz], F32, tag="s", name=f"s_{uname}")
        for j in range(ncz):
            c0 = j * 512
            cw = min(512, CPAD - c0)
            e_scr = epool.tile([128, 512], BF16, tag="e",
                               name=f"e_{uname}_{j}")
            nc.scalar.activation(e_scr[:, :cw], z[:, c0:c0 + cw], AF.Exp,
                                 accum_out=s[:, j:j + 1])
        stot = spool.tile([128, 1], F32, tag="stot", name=f"st_{uname}")
        nc.vector.tensor_reduce(stot[:], s[:], mybir.AxisListType.X,
                                mybir.AluOpType.add)
        if not with_bias:
            nc.vector.tensor_scalar_add(stot[:], stot[:], float(C - CPAD))
        lse = spool.tile([128, 1], F32, tag="lse", name=f"lse_{uname}")
        nc.scalar.activation(lse[:], stot[:], AF.Ln)
        for j in range(ncz):
            c0 = j * 512
            cw = min(512, CPAD - c0)
            nc.vector.tensor_scalar_sub(z[:, c0:c0 + cw], z[:, c0:c0 + cw],
                                        lse[:])
            nc.scalar.dma_start(out_ap[:, c0:c0 + cw], z[:, c0:c0 + cw])


def _build_nc(with_bias=True):
    nc = bacc.Bacc("TRN2", target_bir_lowering=False, debug=False,
                   num_devices=8)

    def din(name, shape, dt=BF16):
        return nc.dram_tensor(name, shape, dt, kind="ExternalInput").ap()

    xw = din("xw", [128, D])
    xq = din("xq", [128, D])
    dt1 = FP8 if "l1" in FP8_LAYERS else BF16
    dt2 = FP8 if "l2" in FP8_LAYERS else BF16
    dt3 = FP8 if "l3" in FP8_LAYERS else BF16
    w1cb = din("w1cb", [6 * 128, NK1 * 512], dt1)
    w2cb = din("w2cb", [6 * 128, NK2 * 512], dt2)
    w3cb = din("w3cb", [10 * 128, NK2 * 512], dt3)
    w3cbl = din("w3cbl", [128, NK2 * 128], dt3)
    w1qcb = din("w1qcb", [6 * 128, NK1 * 512], dt1)
    w2qcb = din("w2qcb", [2 * 128, NK2 * 384], dt2)
    w3qcb = din("w3qcb", [10 * 128, 6 * 512], dt3)
    w3qcbl = din("w3qcbl", [128, 6 * 128], dt3)
    if with_bias:
        b1w = din("b1w", [1, H])
        b2w = din("b2w", [1, H])
        b3w = din("b3w", [1, CPAD])
        b1q = din("b1q", [1, H])
        b2q = din("b2q", [1, QCOLS])
    ones = din("ones", [1, 128])
    ident = din("ident", [128, 128])
    ident8 = din("ident8", [128, 128], FP8)
    outw = nc.dram_tensor("outw", [128, CPAD], F32, kind="ExternalOutput").ap()
    outq = nc.dram_tensor("outq", [128, CPAD], F32, kind="ExternalOutput").ap()

    wbufs = 7 if ("l2" in FP8_LAYERS and "l3" in FP8_LAYERS) else 4
    with tile.TileContext(nc) as tc:
        with tc.tile_pool(name="hp", bufs=5) as hpool, \
             tc.tile_pool(name="wp", bufs=wbufs) as wpool, \
             tc.tile_pool(name="zp", bufs=2) as zpool, \
             tc.tile_pool(name="ep", bufs=2) as epool, \
             tc.tile_pool(name="sp", bufs=2) as spool, \
             tc.tile_pool(name="cp", bufs=1) as cpool, \
             tc.tile_pool(name="ps", bufs=6, space="PSUM") as pspool, \
             tc.tile_pool(name="tp", bufs=2, space="PSUM") as tppool:
            pools = (hpool, wpool, zpool, epool, spool, pspool, tppool)

            # x first (first matmuls need it), consts on the ACT HWDGE queue
            # so they don't delay the weight-block stream on the SP queue
            xw_t = cpool.tile([128, D], BF16, tag="xw")
            nc.sync.dma_start(xw_t[:], xw)
            ones_t = cpool.tile([1, 128], BF16, tag="ones")
            nc.scalar.dma_start(ones_t[:], ones)
            ident_t = cpool.tile([128, 128], BF16, tag="ident")
            nc.scalar.dma_start(ident_t[:], ident)
            xq_t = cpool.tile([128, D], BF16, tag="xq")
            nc.scalar.dma_start(xq_t[:], xq)
            if with_bias:
                b1w_t = cpool.tile([1, H], BF16, tag="b1w")
                nc.scalar.dma_start(b1w_t[:], b1w)
                b2w_t = cpool.tile([1, H], BF16, tag="b2w")
                nc.scalar.dma_start(b2w_t[:], b2w)
                b3w_t = cpool.tile([1, CPAD], BF16, tag="b3w")
                nc.scalar.dma_start(b3w_t[:], b3w)
                b1q_t = cpool.tile([1, H], BF16, tag="b1q")
                nc.scalar.dma_start(b1q_t[:], b1q)
                b2q_t = cpool.tile([1, QCOLS], BF16, tag="b2q")
                nc.scalar.dma_start(b2q_t[:], b2q)
            else:
                b1w_t = b2w_t = b3w_t = b1q_t = b2q_t = None
            ident8_t = cpool.tile([128, 128], FP8, tag="ident8")
            nc.scalar.dma_start(ident8_t[:], ident8)

            _mlp_unit(nc, pools, xw_t[:], w1cb, w2cb, w3cb, w3cbl,
                      b1w_t, b2w_t, b3w_t, ones_t, ident_t, ident8_t, outw,
                      nk3=NK2, ncols2=H, cw2=512, whole=True, uname="w",
                      with_bias=with_bias)
            _mlp_unit(nc, pools, xq_t[:], w1qcb, w2qcb, w3qcb, w3qcbl,
                      b1q_t, b2q_t, None, ones_t, ident_t, ident8_t, outq,
                      nk3=QCOLS // 128, ncols2=QCOLS, cw2=384, whole=False,
                      uname="q", with_bias=with_bias)
    nc.compile()
    return nc


def _cb_pack(W, cw, layer):
    """[K, Ctot] -> per-cw-chunk column blocks [nch*128, nk*cw] where
    block row p, col k*cw + c = W[k*128 + p, j*cw + c]. In DoubleRow mode
    (fp8 l2/l3) rows pair up per 256-super: col t*2cw + i*cw + c maps to
    row 256t + 128i + p."""
    K, Ct = W.shape
    nk, nch = K // 128, Ct // cw
    if layer in FP8_LAYERS:
        ndt = NF8
        Wr = (np.asarray(W, dtype=np.float32) * FP8_SCALE).astype(NF8)
    else:
        ndt = NBF
        Wr = np.asarray(W, dtype=NBF)
    Wr = Wr.reshape(nk, 128, Ct)
    out = np.empty((nch * 128, nk * cw), dtype=ndt)
    for j in range(nch):
        blk = Wr[:, :, j * cw:(j + 1) * cw]        # [nk, 128, cw]
        if MOE_DR and layer in ("l2", "l3"):
            # [t, i, p, c] -> [p, t, i, c] -> cols ordered t*2cw + i*cw + c
            out[j * 128:(j + 1) * 128] = (
                blk.reshape(nk // 2, 2, 128, cw).transpose(2, 0, 1, 3)
                .reshape(128, nk * cw))
        else:
            out[j * 128:(j + 1) * 128] = (
                blk.transpose(1, 0, 2).reshape(128, nk * cw))
    return out


def _erf(v):
    try:
        from scipy.special import erf
        return erf(v)
    except ImportError:
        import math
        return np.vectorize(math.erf)(v)


def _host_expert(x_rows, W1e, b1e, W2e, b2e, W3e, b3e):
    """fp32 numpy fallback, mirroring the reference exactly."""

    def gelu(v):
        return (v * 0.5 * (1.0 + _erf(v / np.sqrt(2.0)))).astype(np.float32)

    h1 = gelu(x_rows @ W1e + b1e)
    h2 = gelu(h1 @ W2e + b2e)
    z = (h2 @ W3e + b3e).astype(np.float64)
    m = z.max(axis=1, keepdims=True)
    lse = np.log(np.exp(z - m).sum(axis=1, keepdims=True)) + m
    return (z - lse).astype(np.float32)


def kernel(x, mask_frac, W1, b1, W2, b2, W3, b3):
    global LAST_RESULTS, _NC_CACHE

    x = np.asarray(x, dtype=np.float32)
    mask_frac = np.asarray(mask_frac, dtype=np.float32)
    W1 = np.asarray(W1, dtype=np.float32)
    b1 = np.asarray(b1, dtype=np.float32)
    W2 = np.asarray(W2, dtype=np.float32)
    b2 = np.asarray(b2, dtype=np.float32)
    W3 = np.asarray(W3, dtype=np.float32)
    b3 = np.asarray(b3, dtype=np.float32)

    # host routing, mirroring the reference's fp32 arithmetic
    t = np.float32(1.0) - mask_frac
    bins = (t / np.float32(0.1)).astype(np.int32)

    with_bias = bool(b1.any() or b2.any() or b3.any())

    groups = [np.where(bins == e)[0] for e in range(E)]
    fallback = []  # (expert, sample indices) pairs computed on host
    dev_groups = []
    for e in range(10):
        idx = groups[e]
        if len(idx) > CAP:
            fallback.append((e, idx[CAP:]))
            idx = idx[:CAP]
        dev_groups.append(idx)
    if len(groups[10]):
        fallback.append((10, groups[10]))

    def pack_x(idx):
        # [128, D] bf16 with xs[p, k*128 + n] = x[idx[n], k*128 + p]
        xt = np.zeros((128, D), dtype=NBF)
        if len(idx):
            xe = x[idx].astype(NBF)            # [n, D]
            xr = np.ascontiguousarray(
                xe.reshape(len(idx), NK1, 128).transpose(2, 1, 0))
            xt.reshape(128, NK1, 128)[:, :, :len(idx)] = xr
        return xt

    bsc1 = FP8_SCALE if "l1" in FP8_LAYERS else 1.0
    bsc2 = FP8_SCALE if "l2" in FP8_LAYERS else 1.0
    bsc3 = FP8_SCALE if "l3" in FP8_LAYERS else 1.0
    b3pad = np.full((1, CPAD), PAD_BIAS * bsc3, dtype=NBF)
    ones_np = np.ones((1, 128), dtype=NBF)
    ident_np = np.eye(128, dtype=NBF)
    ident8_np = np.eye(128, dtype=NF8)

    in_maps = []
    for c in range(8):
        q = 8 if c < 4 else 9          # split expert handled by this core
        qq = c % 4                     # hidden-dim quarter index
        b3row = b3pad.copy()
        b3row[0, :C] = (b3[c] * bsc3).astype(NBF)
        w3pad = np.zeros((H, CPAD), dtype=np.float32)
        w3pad[:, :C] = W3[c]
        w3qpad = np.zeros((QCOLS, CPAD), dtype=np.float32)
        w3qpad[:, :C] = W3[q][qq * QCOLS:(qq + 1) * QCOLS]
        bias_ins = {
            "b1w": (b1[c] * bsc1).astype(NBF).reshape(1, H),
            "b2w": (b2[c] * bsc2).astype(NBF).reshape(1, H),
            "b3w": b3row,
            "b1q": (b1[q] * bsc1).astype(NBF).reshape(1, H),
            "b2q": np.ascontiguousarray(
                (b2[q][qq * QCOLS:(qq + 1) * QCOLS] * bsc2).astype(NBF)
            ).reshape(1, QCOLS),
        } if with_bias else {}
        in_maps.append({
            **bias_ins,
            "xw": pack_x(dev_groups[c]),
            "xq": pack_x(dev_groups[q]),
            "w1cb": _cb_pack(W1[c], 512, "l1"),
            "w2cb": _cb_pack(W2[c], 512, "l2"),
            "w3cb": _cb_pack(w3pad[:, :CMAIN], 512, "l3"),
            "w3cbl": _cb_pack(w3pad[:, CMAIN:], 128, "l3"),
            "w1qcb": _cb_pack(W1[q], 512, "l1"),
            "w2qcb": _cb_pack(W2[q][:, qq * QCOLS:(qq + 1) * QCOLS], 384, "l2"),
            "w3qcb": _cb_pack(w3qpad[:, :CMAIN], 512, "l3"),
            "w3qcbl": _cb_pack(w3qpad[:, CMAIN:], 128, "l3"),
            "ones": ones_np,
            "ident": ident_np,
            "ident8": ident8_np,
        })

    if with_bias not in _NC_CACHE:
        _NC_CACHE[with_bias] = _build_nc(with_bias)
    res = run_bass_kernel_spmd(_NC_CACHE[with_bias], in_maps,
                               core_ids=list(range(8)))
    LAST_RESULTS = res

    out = np.zeros((B, C), dtype=np.float32)
    for c in range(8):
        idx = dev_groups[c]
        if len(idx):
            out[idx] = res.results[c]["outw"][:len(idx), :C]

    # split experts: host-sum the 4 hidden-quarter partials + b3, log_softmax
    for q, cores in ((8, (0, 1, 2, 3)), (9, (4, 5, 6, 7))):
        idx = dev_groups[q]
        if not len(idx):
            continue
        zsum = np.zeros((len(idx), C), dtype=np.float64)
        for c in cores:
            zsum += res.results[c]["outq"][:len(idx), :C]
        zsum += b3[q]
        m = zsum.max(axis=1, keepdims=True)
        lse = np.log(np.exp(zsum - m).sum(axis=1, keepdims=True)) + m
        out[idx] = (zsum - lse).astype(np.float32)

    for e, idx in fallback:
        out[idx] = _host_expert(x[idx], W1[e], b1[e], W2[e], b2[e],
                                W3[e], b3[e])
    return out


# revision 22
# speedup vs baseline: 2.0566x; 1.1975x over previous
"""Trainium2 Bass kernel for the time-binned MoE EmbeddingClassifier.

Model: 11 expert MLPs (1536 -> 3072 -> 3072 -> 5242, exact GELU between
layers, log_softmax output). Each sample is routed to one expert by
bin = trunc((1 - mask_frac) / 0.1).

Strategy (8 NeuronCores, expert-parallel with host-side routing):
  - Routing is computed on the host from mask_frac; samples are grouped by
    expert. Only the routed expert runs per sample (11x less compute than
    the reference's run-all-then-select).
  - Experts 0..7 are whole-expert assigned to cores 0..7.
  - Experts 8 and 9 are each split 4 ways along the hidden dimension
    (cores 0-3 handle expert 8, cores 4-7 handle expert 9): each core
    computes the full layer 1, a 768-column slice of layer 2, and the
    matching 768-row slice of layer 3, producing a full-width partial
    logit sum. The host adds the 4 partials + b3 and applies log_softmax.
    This balances HBM weight traffic at ~83 MB/core (vs 121 MB for a
    naive 2-experts-on-one-core split).
  - Precision: x / W1 / activations in fp16 (same HBM bytes and PE rate
    as bf16 but 8x finer mantissa on this small-range data); W2/W3 in
    e4m3 fp8 with a x64 power-of-2 pre-scale (|W|~0.02 sits in e4m3's
    subnormal range) and the descale folded into the existing
    PSUM-drain ACT ops. Accumulation stays fp32 in PSUM; the logits and
    log_softmax stay fp32. Measured vs the fp32 reference:
    L2 rel err ~1.6e-3, absmax ~0.07 (vs |out| ~ 9-15).
  - Expert 10 (hit only when mask_frac == 0.0 exactly) and any samples
    beyond the per-expert capacity of 128 are computed on the host in
    fp32 as a correctness fallback; for the expected input distribution
    (~102 samples/expert) this never triggers.

Device layout: activations ride the partition dim as [samples<=128, feat];
weights stream as the moving matmul operand. Weights are host-packed into
per-output-chunk column blocks ([128, nk*cw] per 512-wide output chunk) so
the k-loop accumulates into a single PSUM bank with back-to-back matmuls
(no PSUM bank cycling -> PE stays pipelined and HAM-warm), and each block
arrives via ~1 MB DMA pieces so the PE never starves past the ~3.4 us HAM
re-throttle window. Between layers the activations are transposed 128x128
via the PE.
"""

import os
import sys

if "/opt/trn_rl_repo" not in sys.path:
    sys.path.insert(0, "/opt/trn_rl_repo")

import numpy as np
import ml_dtypes

import concourse.bass as bass
import concourse.tile as tile
from concourse import bacc, mybir
from concourse.bass_utils import run_bass_kernel_spmd

# half dtype for x / non-fp8 weights / activations: fp16 beats bf16 here --
# same bytes and PE rate, but 8x finer mantissa on this small-range data
MOE_HALF = os.environ.get("MOE_HALF", "fp16")
BF16 = mybir.dt.float16 if MOE_HALF == "fp16" else mybir.dt.bfloat16
FP8 = mybir.dt.float8e4
F32 = mybir.dt.float32
AF = mybir.ActivationFunctionType
NBF = np.float16 if MOE_HALF == "fp16" else ml_dtypes.bfloat16
NF8 = ml_dtypes.float8_e4m3
FP8_SCALE = 64.0     # power-of-2 pre-scale: |W|~0.02 sits in e4m3's subnormal
                     # range, x64 recenters it; descale rides the ACT op
# which layers stream fp8 weights ("l1"/"l2"/"l3"); overridable for A/B runs
FP8_LAYERS = frozenset(os.environ.get("MOE_FP8", "l2,l3").replace(",", " ").split())
# DoubleRow perf mode for the fp8 layers 2/3 (needs fp8 activations too)
MOE_DR = os.environ.get("MOE_DR", "0") == "1"
# transpose path: "pe" (tensor-engine + DVE drain) or "dma" (xbar transpose)
MOE_TR = os.environ.get("MOE_TR", "pe")
DRMODE = mybir.MatmulPerfMode.DoubleRow

E = 11
D = 1536
H = 3072
C = 5242
B = 1024
CAP = 128            # per-expert sample capacity on device
CPAD = 5248          # C padded to a multiple of 128 (10x512 + 128)
CMAIN = 5120         # first 10 layer-3 chunks (512 wide)
NK1 = D // 128       # 12 k-tiles for layer 1
NK2 = H // 128       # 24 k-tiles for layers 2/3
QCOLS = H // 4       # 768-wide hidden slice for the split experts
PAD_BIAS = -100.0    # b3 value for padded logit columns -> exp() == 0

LAST_RESULTS = None  # BassKernelResults of the most recent run (for test.py)

_NC_CACHE = {}


def _chunk_mm(nc, wpool, pspool, lhs_full, nk, wdram, jrow, cw, npieces,
              name, wdt=BF16, final_stop=False, dr=False):
    """Accumulate one [128, cw] output chunk over nk k-tiles into one PSUM
    tile. Weight block [128, nk*cw] is DMAed in npieces k-contiguous pieces
    (subtile deps let early matmuls start before the whole block lands)."""
    psum = pspool.tile([128, 512], F32, tag="acc", name=f"ps_{name}")
    wblk = wpool.tile([128, nk * cw], wdt, tag="wblk", name=f"wb_{name}")
    cols = nk * cw
    kg = nk // npieces
    for pc in range(npieces):
        c0 = pc * kg * cw
        c1 = cols if pc == npieces - 1 else (pc + 1) * kg * cw
        nc.sync.dma_start(wblk[:, c0:c1],
                          wdram[jrow * 128:(jrow + 1) * 128, c0:c1])
    if dr:
        for t in range(nk // 2):
            lhs = lhs_full[:, 256 * t:256 * (t + 1)].rearrange(
                "p (i m) -> p i m", i=2)
            rhs = wblk[:, 2 * cw * t:2 * cw * (t + 1)].rearrange(
                "p (i n) -> p i n", i=2)
            nc.tensor.matmul(psum[:, :cw], lhs, rhs, perf_mode=DRMODE,
                             start=(t == 0),
                             stop=(final_stop and t == nk // 2 - 1))
    else:
        for k in range(nk):
            nc.tensor.matmul(psum[:, :cw], lhs_full[:, k * 128:(k + 1) * 128],
                             wblk[:, k * cw:(k + 1) * cw],
                             start=(k == 0),
                             stop=(final_stop and k == nk - 1))
    return psum


def _transpose(nc, hpool, tppool, src, ncols, ident_t, name, hdt=BF16):
    """Transpose src [128, ncols] per 128-chunk -> new tile [128, ncols].
    MOE_TR="pe": tensor-engine transpose (bf16; fp8 needs stride-2 PSUM APs)
    + DVE drain-copy casting to hdt. MOE_TR="dma": xbar DMA transpose
    (2-byte dtypes only, so hdt must not be fp8)."""
    out = hpool.tile([128, H], hdt, tag="h", name=f"t_{name}")
    if MOE_TR == "dma" and hdt is not FP8:
        for k in range(ncols // 128):
            nc.scalar.dma_start_transpose(out[:, k * 128:(k + 1) * 128],
                                          src[:, k * 128:(k + 1) * 128])
        return out
    for k in range(ncols // 128):
        tp = tppool.tile([128, 128], BF16, tag="tp", name=f"tp_{name}_{k}")
        nc.tensor.transpose(tp[:], src[:, k * 128:(k + 1) * 128], ident_t[:])
        nc.vector.tensor_copy(out[:, k * 128:(k + 1) * 128], tp[:])
    return out


def _mlp_unit(nc, pools, xs, w1cb, w2cb, w3cb, w3cbl, b1s, b2s, b3s, ones_t,
              ident_t, ident8_t, out_ap, nk3, ncols2, cw2, whole, uname,
              with_bias=True):
    """One expert unit: x -> gelu -> gelu -> logits [-> log_softmax] -> out."""
    hpool, wpool, zpool, epool, spool, pspool, tppool = pools
    dt1 = FP8 if "l1" in FP8_LAYERS else BF16
    dt2 = FP8 if "l2" in FP8_LAYERS else BF16
    dt3 = FP8 if "l3" in FP8_LAYERS else BF16
    sc1 = 1.0 / FP8_SCALE if dt1 is FP8 else 1.0
    sc2 = 1.0 / FP8_SCALE if dt2 is FP8 else 1.0
    sc3 = 1.0 / FP8_SCALE if dt3 is FP8 else 1.0
    # DoubleRow: fp8 weights AND fp8 activations for layers 2/3
    hdt = FP8 if MOE_DR else BF16
    tid = ident_t

    # ---- layer 1: h1[s, h] = gelu(x @ W1 + b1), 6 chunks of 512
    h1 = hpool.tile([128, H], BF16, tag="h", name=f"h1_{uname}")
    for j in range(H // 512):
        ps = _chunk_mm(nc, wpool, pspool, xs, NK1, w1cb, j, 512, 2,
                       f"{uname}l1j{j}", wdt=dt1, final_stop=not with_bias)
        if with_bias:
            nc.tensor.matmul(ps[:], ones_t[:], b1s[:, j * 512:(j + 1) * 512],
                             start=False, stop=True)
        nc.scalar.activation(h1[:, j * 512:(j + 1) * 512], ps[:], AF.Gelu,
                             scale=sc1)

    h1t = _transpose(nc, hpool, tppool, h1, H, tid, f"h1_{uname}", hdt=hdt)

    # ---- layer 2: h2[s, c] = gelu(h1 @ W2 + b2) over ncols2 cols
    h2 = hpool.tile([128, H], BF16, tag="h", name=f"h2_{uname}")
    for j in range(ncols2 // cw2):
        ps = _chunk_mm(nc, wpool, pspool, h1t, NK2, w2cb, j, cw2, 3,
                       f"{uname}l2j{j}", wdt=dt2, dr=MOE_DR,
                       final_stop=not with_bias)
        if with_bias:
            nc.tensor.matmul(ps[:, :cw2], ones_t[:],
                             b2s[:, j * cw2:(j + 1) * cw2],
                             start=False, stop=True)
        nc.scalar.activation(h2[:, j * cw2:(j + 1) * cw2], ps[:, :cw2],
                             AF.Gelu, scale=sc2)

    h2t = _transpose(nc, hpool, tppool, h2, ncols2, tid, f"h2_{uname}", hdt=hdt)

    # ---- layer 3: z[s, c] = h2 @ W3 (+ b3), 10x512 + 1x128 chunks
    z = zpool.tile([128, CPAD], F32, tag="z", name=f"z_{uname}")
    l3 = [(j, j * 512, 512, w3cb, j, 3 if nk3 == NK2 else 1)
          for j in range(CMAIN // 512)]
    l3.append((10, CMAIN, 128, w3cbl, 0, 1))
    for j, c0, cw, wdram, jrow, npieces in l3:
        ps = _chunk_mm(nc, wpool, pspool, h2t, nk3, wdram, jrow, cw, npieces,
                       f"{uname}l3j{j}", wdt=dt3,
                       final_stop=(not whole) or (not with_bias), dr=MOE_DR)
        if whole and with_bias:
            nc.tensor.matmul(ps[:, :cw], ones_t[:], b3s[:, c0:c0 + cw],
                             start=False, stop=True)
        nc.scalar.activation(z[:, c0:c0 + cw], ps[:, :cw], AF.Copy,
                             bias=0.0, scale=sc3)
        if not whole:
            nc.scalar.dma_start(out_ap[:, c0:c0 + cw], z[:, c0:c0 + cw])

    if whole:
        # ---- log_softmax along the free axis (pad cols hold z = -100)
        ncz = CPAD // 512 + 1
        s = spool.tile([128, ncz], F32, tag="s", name=f"s_{uname}")
        for j in range(ncz):
            c0 = j * 512
            cw = min(512, CPAD - c0)
            e_scr = epool.tile([128, 512], BF16, tag="e",
                               name=f"e_{uname}_{j}")
            nc.scalar.activation(e_scr[:, :cw], z[:, c0:c0 + cw], AF.Exp,
                                 accum_out=s[:, j:j + 1])
        stot = spool.tile([128, 1], F32, tag="stot", name=f"st_{uname}")
        nc.vector.tensor_reduce(stot[:], s[:], mybir.AxisListType.X,
                                mybir.AluOpType.add)
        if not with_bias:
            nc.vector.tensor_scalar_add(stot[:], stot[:], float(C - CPAD))
        lse = spool.tile([128, 1], F32, tag="lse", name=f"lse_{uname}")
        nc.scalar.activation(lse[:], stot[:], AF.Ln)
        for j in range(ncz):
            c0 = j * 512
            cw = min(512, CPAD - c0)
            nc.vector.tensor_scalar_sub(z[:, c0:c0 + cw], z[:, c0:c0 + cw],
                                        lse[:])
            nc.scalar.dma_start(out_ap[:, c0:c0 + cw], z[:, c0:c0 + cw])


def _build_nc(with_bias=True):
    nc = bacc.Bacc("TRN2", target_bir_lowering=False, debug=False,
                   num_devices=8)

    def din(name, shape, dt=BF16):
        return nc.dram_tensor(name, shape, dt, kind="ExternalInput").ap()

    xw = din("xw", [128, D])
    xq = din("xq", [128, D])
    dt1 = FP8 if "l1" in FP8_LAYERS else BF16
    dt2 = FP8 if "l2" in FP8_LAYERS else BF16
    dt3 = FP8 if "l3" in FP8_LAYERS else BF16
    w1cb = din("w1cb", [6 * 128, NK1 * 512], dt1)
    w2cb = din("w2cb", [6 * 128, NK2 * 512], dt2)
    w3cb = din("w3cb", [10 * 128, NK2 * 512], dt3)
    w3cbl = din("w3cbl", [128, NK2 * 128], dt3)
    w1qcb = din("w1qcb", [6 * 128, NK1 * 512], dt1)
    w2qcb = din("w2qcb", [2 * 128, NK2 * 384], dt2)
    w3qcb = din("w3qcb", [10 * 128, 6 * 512], dt3)
    w3qcbl = din("w3qcbl", [128, 6 * 128], dt3)
    if with_bias:
        b1w = din("b1w", [1, H])
        b2w = din("b2w", [1, H])
        b3w = din("b3w", [1, CPAD])
        b1q = din("b1q", [1, H])
        b2q = din("b2q", [1, QCOLS])
    ones = din("ones", [1, 128])
    ident = din("ident", [128, 128])
    ident8 = din("ident8", [128, 128], FP8)
    outw = nc.dram_tensor("outw", [128, CPAD], F32, kind="ExternalOutput").ap()
    outq = nc.dram_tensor("outq", [128, CPAD], F32, kind="ExternalOutput").ap()

    wbufs = 7 if ("l2" in FP8_LAYERS and "l3" in FP8_LAYERS) else 4
    with tile.TileContext(nc) as tc:
        with tc.tile_pool(name="hp", bufs=5) as hpool, \
             tc.tile_pool(name="wp", bufs=wbufs) as wpool, \
             tc.tile_pool(name="zp", bufs=2) as zpool, \
             tc.tile_pool(name="ep", bufs=2) as epool, \
             tc.tile_pool(name="sp", bufs=2) as spool, \
             tc.tile_pool(name="cp", bufs=1) as cpool, \
             tc.tile_pool(name="ps", bufs=5, space="PSUM") as pspool, \
             tc.tile_pool(name="tp", bufs=3, space="PSUM") as tppool:
            pools = (hpool, wpool, zpool, epool, spool, pspool, tppool)

            # x first (first matmuls need it), consts on the ACT HWDGE queue
            # so they don't delay the weight-block stream on the SP queue
            xw_t = cpool.tile([128, D], BF16, tag="xw")
            nc.sync.dma_start(xw_t[:], xw)
            ones_t = cpool.tile([1, 128], BF16, tag="ones")
            nc.scalar.dma_start(ones_t[:], ones)
            ident_t = cpool.tile([128, 128], BF16, tag="ident")
            nc.scalar.dma_start(ident_t[:], ident)
            xq_t = cpool.tile([128, D], BF16, tag="xq")
            nc.scalar.dma_start(xq_t[:], xq)
            if with_bias:
                b1w_t = cpool.tile([1, H], BF16, tag="b1w")
                nc.scalar.dma_start(b1w_t[:], b1w)
                b2w_t = cpool.tile([1, H], BF16, tag="b2w")
                nc.scalar.dma_start(b2w_t[:], b2w)
                b3w_t = cpool.tile([1, CPAD], BF16, tag="b3w")
                nc.scalar.dma_start(b3w_t[:], b3w)
                b1q_t = cpool.tile([1, H], BF16, tag="b1q")
                nc.scalar.dma_start(b1q_t[:], b1q)
                b2q_t = cpool.tile([1, QCOLS], BF16, tag="b2q")
                nc.scalar.dma_start(b2q_t[:], b2q)
            else:
                b1w_t = b2w_t = b3w_t = b1q_t = b2q_t = None
            ident8_t = cpool.tile([128, 128], FP8, tag="ident8")
            nc.scalar.dma_start(ident8_t[:], ident8)

            _mlp_unit(nc, pools, xw_t[:], w1cb, w2cb, w3cb, w3cbl,
                      b1w_t, b2w_t, b3w_t, ones_t, ident_t, ident8_t, outw,
                      nk3=NK2, ncols2=H, cw2=512, whole=True, uname="w",
                      with_bias=with_bias)
            _mlp_unit(nc, pools, xq_t[:], w1qcb, w2qcb, w3qcb, w3qcbl,
                      b1q_t, b2q_t, None, ones_t, ident_t, ident8_t, outq,
                      nk3=QCOLS // 128, ncols2=QCOLS, cw2=384, whole=False,
                      uname="q", with_bias=with_bias)
    nc.compile()
    return nc


def _cb_pack(W, cw, layer):
    """[K, Ctot] -> per-cw-chunk column blocks [nch*128, nk*cw] where
    block row p, col k*cw + c = W[k*128 + p, j*cw + c]. In DoubleRow mode
    (fp8 l2/l3) rows pair up per 256-super: col t*2cw + i*cw + c maps to
    row 256t + 128i + p."""
    K, Ct = W.shape
    nk, nch = K // 128, Ct // cw
    if layer in FP8_LAYERS:
        ndt = NF8
        Wr = (np.asarray(W, dtype=np.float32) * FP8_SCALE).astype(NF8)
    else:
        ndt = NBF
        Wr = np.asarray(W, dtype=NBF)
    Wr = Wr.reshape(nk, 128, Ct)
    out = np.empty((nch * 128, nk * cw), dtype=ndt)
    for j in range(nch):
        blk = Wr[:, :, j * cw:(j + 1) * cw]        # [nk, 128, cw]
        if MOE_DR and layer in ("l2", "l3"):
            # [t, i, p, c] -> [p, t, i, c] -> cols ordered t*2cw + i*cw + c
            out[j * 128:(j + 1) * 128] = (
                blk.reshape(nk // 2, 2, 128, cw).transpose(2, 0, 1, 3)
                .reshape(128, nk * cw))
        else:
            out[j * 128:(j + 1) * 128] = (
                blk.transpose(1, 0, 2).reshape(128, nk * cw))
    return out


def _erf(v):
    try:
        from scipy.special import erf
        return erf(v)
    except ImportError:
        import math
        return np.vectorize(math.erf)(v)


def _host_expert(x_rows, W1e, b1e, W2e, b2e, W3e, b3e):
    """fp32 numpy fallback, mirroring the reference exactly."""

    def gelu(v):
        return (v * 0.5 * (1.0 + _erf(v / np.sqrt(2.0)))).astype(np.float32)

    h1 = gelu(x_rows @ W1e + b1e)
    h2 = gelu(h1 @ W2e + b2e)
    z = (h2 @ W3e + b3e).astype(np.float64)
    m = z.max(axis=1, keepdims=True)
    lse = np.log(np.exp(z - m).sum(axis=1, keepdims=True)) + m
    return (z - lse).astype(np.float32)


def kernel(x, mask_frac, W1, b1, W2, b2, W3, b3):
    global LAST_RESULTS, _NC_CACHE

    x = np.asarray(x, dtype=np.float32)
    mask_frac = np.asarray(mask_frac, dtype=np.float32)
    W1 = np.asarray(W1, dtype=np.float32)
    b1 = np.asarray(b1, dtype=np.float32)
    W2 = np.asarray(W2, dtype=np.float32)
    b2 = np.asarray(b2, dtype=np.float32)
    W3 = np.asarray(W3, dtype=np.float32)
    b3 = np.asarray(b3, dtype=np.float32)

    # host routing, mirroring the reference's fp32 arithmetic
    t = np.float32(1.0) - mask_frac
    bins = (t / np.float32(0.1)).astype(np.int32)

    with_bias = bool(b1.any() or b2.any() or b3.any())

    groups = [np.where(bins == e)[0] for e in range(E)]
    fallback = []  # (expert, sample indices) pairs computed on host
    dev_groups = []
    for e in range(10):
        idx = groups[e]
        if len(idx) > CAP:
            fallback.append((e, idx[CAP:]))
            idx = idx[:CAP]
        dev_groups.append(idx)
    if len(groups[10]):
        fallback.append((10, groups[10]))

    def pack_x(idx):
        # [128, D] bf16 with xs[p, k*128 + n] = x[idx[n], k*128 + p]
        xt = np.zeros((128, D), dtype=NBF)
        if len(idx):
            xe = x[idx].astype(NBF)            # [n, D]
            xr = np.ascontiguousarray(
                xe.reshape(len(idx), NK1, 128).transpose(2, 1, 0))
            xt.reshape(128, NK1, 128)[:, :, :len(idx)] = xr
        return xt

    bsc1 = FP8_SCALE if "l1" in FP8_LAYERS else 1.0
    bsc2 = FP8_SCALE if "l2" in FP8_LAYERS else 1.0
    bsc3 = FP8_SCALE if "l3" in FP8_LAYERS else 1.0
    b3pad = np.full((1, CPAD), PAD_BIAS * bsc3, dtype=NBF)
    ones_np = np.ones((1, 128), dtype=NBF)
    ident_np = np.eye(128, dtype=NBF)
    ident8_np = np.eye(128, dtype=NF8)

    in_maps = []
    for c in range(8):
        q = 8 if c < 4 else 9          # split expert handled by this core
        qq = c % 4                     # hidden-dim quarter index
        b3row = b3pad.copy()
        b3row[0, :C] = (b3[c] * bsc3).astype(NBF)
        w3pad = np.zeros((H, CPAD), dtype=np.float32)
        w3pad[:, :C] = W3[c]
        w3qpad = np.zeros((QCOLS, CPAD), dtype=np.float32)
        w3qpad[:, :C] = W3[q][qq * QCOLS:(qq + 1) * QCOLS]
        bias_ins = {
            "b1w": (b1[c] * bsc1).astype(NBF).reshape(1, H),
            "b2w": (b2[c] * bsc2).astype(NBF).reshape(1, H),
            "b3w": b3row,
            "b1q": (b1[q] * bsc1).astype(NBF).reshape(1, H),
            "b2q": np.ascontiguousarray(
                (b2[q][qq * QCOLS:(qq + 1) * QCOLS] * bsc2).astype(NBF)
            ).reshape(1, QCOLS),
        } if with_bias else {}
        in_maps.append({
            **bias_ins,
            "xw": pack_x(dev_groups[c]),
            "xq": pack_x(dev_groups[q]),
            "w1cb": _cb_pack(W1[c], 512, "l1"),
            "w2cb": _cb_pack(W2[c], 512, "l2"),
            "w3cb": _cb_pack(w3pad[:, :CMAIN], 512, "l3"),
            "w3cbl": _cb_pack(w3pad[:, CMAIN:], 128, "l3"),
            "w1qcb": _cb_pack(W1[q], 512, "l1"),
            "w2qcb": _cb_pack(W2[q][:, qq * QCOLS:(qq + 1) * QCOLS], 384, "l2"),
            "w3qcb": _cb_pack(w3qpad[:, :CMAIN], 512, "l3"),
            "w3qcbl": _cb_pack(w3qpad[:, CMAIN:], 128, "l3"),
            "ones": ones_np,
            "ident": ident_np,
            "ident8": ident8_np,
        })

    if with_bias not in _NC_CACHE:
        _NC_CACHE[with_bias] = _build_nc(with_bias)
    res = run_bass_kernel_spmd(_NC_CACHE[with_bias], in_maps,
                               core_ids=list(range(8)))
    LAST_RESULTS = res

    out = np.zeros((B, C), dtype=np.float32)
    for c in range(8):
        idx = dev_groups[c]
        if len(idx):
            out[idx] = res.results[c]["outw"][:len(idx), :C]

    # split experts: host-sum the 4 hidden-quarter partials + b3, log_softmax
    for q, cores in ((8, (0, 1, 2, 3)), (9, (4, 5, 6, 7))):
        idx = dev_groups[q]
        if not len(idx):
            continue
        zsum = np.zeros((len(idx), C), dtype=np.float64)
        for c in cores:
            zsum += res.results[c]["outq"][:len(idx), :C]
        zsum += b3[q]
        m = zsum.max(axis=1, keepdims=True)
        lse = np.log(np.exp(zsum - m).sum(axis=1, keepdims=True)) + m
        out[idx] = (zsum - lse).astype(np.float32)

    for e, idx in fallback:
        out[idx] = _host_expert(x[idx], W1[e], b1[e], W2[e], b2[e],
                                W3[e], b3[e])
    return out
